# revision 16
# baseline (speedup 1.0000x reference)
"""Canny filter Bass kernel for Trainium2, data-parallel over batch on 8 cores.

Layout per core: image (3, 1024, 1024) processed in 8 row-blocks of 128 rows
(partition dim = rows, free dim = columns).  Vertical 3-tap stencils (t, u,
hysteresis box) run on the tensor engine as banded matmuls with cross-block
halo rows folded in via K=2 accumulating matmuls.  The NMS vertical neighbor
maps (q shifted by one row) are produced by SBUF-to-SBUF DMA partition
shifts instead of matmuls.  Horizontal stencils are shifted-view vector ops.
Thresholds and hysteresis gating use fused scalar_tensor_tensor ops.  The
sqrt output phase is data-gated behind the last arctan so the scalar engine
loads each activation table set exactly once.
"""

import os
import tempfile
from contextlib import ExitStack

import numpy as np
import ml_dtypes

import concourse.bacc as bacc
import concourse.tile as tile
from concourse import mybir
from concourse.bass_utils import run_bass_kernel_spmd

F32 = mybir.dt.float32
I32 = mybir.dt.int32
U8 = mybir.dt.uint8
BF16 = mybir.dt.bfloat16
AF = mybir.ActivationFunctionType
ALU = mybir.AluOpType

H = W = 1024
C = 3
NB = 8          # row blocks
P = 128         # rows per block
HALF = 512      # fp32 matmul max moving free dim
INV3 = float(np.float32(1.0) / np.float32(3.0))
K8PI = float(np.float32(8.0 / np.pi))
BIG = 1e18


def _const_weights():
    """f32 [128, 5*128]: Vs/3 | Vd/3 | I | hvT2/3 | hvU2/3.

    Vs = vertical [0.5,1,0.5] band, Vd = vertical [-1,0,1] band, I = identity
    (channel sum).  The sobel /C normalization is folded into the weights.
    hv*2 blocks live in rows 0..1: row 0 multiplies prev-block row 127 (adds
    into out row 0), row 1 multiplies next-block row 0 (adds into out row
    127).
    """
    cw = np.zeros((P, 5 * P), np.float32)

    def blk(i):
        return cw[:, i * P:(i + 1) * P]

    Vs, Vd, I = blk(0), blk(1), blk(2)
    third = np.float32(INV3)
    half3 = np.float32(0.5) * third
    for m in range(P):
        Vs[m, m] = third
        if m > 0:
            Vs[m - 1, m] = half3
            Vd[m - 1, m] = -third
        if m < P - 1:
            Vs[m + 1, m] = half3
            Vd[m + 1, m] = third
        I[m, m] = 1.0
    blk(3)[0, 0] = half3       # t halo
    blk(3)[1, P - 1] = half3
    blk(4)[0, 0] = -third      # u halo
    blk(4)[1, P - 1] = third
    return cw


def _const_weights_bf16():
    """bf16 [128, 2*128]: T3 (vertical ones band) | hvS2."""
    cwb = np.zeros((P, 2 * P), np.float32)
    T3 = cwb[:, 0:P]
    for m in range(P):
        T3[m, m] = 1.0
        if m > 0:
            T3[m - 1, m] = 1.0
        if m < P - 1:
            T3[m + 1, m] = 1.0
    cwb[0, P] = 1.0          # hvS2 row 0
    cwb[1, 2 * P - 1] = 1.0  # hvS2 row 1
    return cwb.astype(ml_dtypes.bfloat16)


def _emit(nc, tc, img, cw, cwb, zs, o_gx, o_gy, o_gm, o_or, o_te):
    v = nc.vector
    sc = nc.scalar
    te = nc.tensor
    gp = nc.gpsimd

    ctx = ExitStack()
    cpool = ctx.enter_context(tc.tile_pool(name="cp", bufs=1))
    inp = ctx.enter_context(tc.tile_pool(name="inp", bufs=1))
    spool = ctx.enter_context(tc.tile_pool(name="sp", bufs=2))
    tupool = ctx.enter_context(tc.tile_pool(name="tu", bufs=2))
    gout = ctx.enter_context(tc.tile_pool(name="go", bufs=2))
    sq = ctx.enter_context(tc.tile_pool(name="sq", bufs=1))
    scr = ctx.enter_context(tc.tile_pool(name="scr", bufs=1))
    qpool = ctx.enter_context(tc.tile_pool(name="qp", bufs=NB))
    shp = ctx.enter_context(tc.tile_pool(name="shp", bufs=2))
    nms = ctx.enter_context(tc.tile_pool(name="nms", bufs=1))
    rpool = ctx.enter_context(tc.tile_pool(name="rp", bufs=2))
    pdpool = ctx.enter_context(tc.tile_pool(name="pd", bufs=2))
    hpool = ctx.enter_context(tc.tile_pool(name="hp", bufs=3))
    outp = ctx.enter_context(tc.tile_pool(name="op", bufs=2))
    psA = ctx.enter_context(tc.tile_pool(name="psA", bufs=2, space="PSUM"))
    psS = ctx.enter_context(tc.tile_pool(name="psS", bufs=1, space="PSUM"))

    cwt = cpool.tile([P, 5 * P], F32, tag="cw")
    nc.sync.dma_start(cwt[:], cw[:])
    cwbt = cpool.tile([P, 2 * P], BF16, tag="cwb")
    nc.sync.dma_start(cwbt[:], cwb[:])

    def wblk(i, rows=P):
        return cwt[0:rows, i * P:(i + 1) * P]

    Vs, Vd, Iw = wblk(0), wblk(1), wblk(2)
    hvT2, hvU2 = wblk(3, 2), wblk(4, 2)
    T3 = cwbt[:, 0:P]
    hvS2 = cwbt[0:2, P:2 * P]

    def conv(out_ps, parts):
        """Accumulating matmuls into out_ps [128, W]; parts = [(lhsT, rhs)]."""
        n = len(parts)
        for i, (lt, rh) in enumerate(parts):
            for h in (0, HALF):
                te.matmul(out_ps[:, h:h + HALF], lt, rh[:, h:h + HALF],
                          start=(i == 0), stop=(i == n - 1))

    s_sb = [None] * NB
    q_sb = [None] * NB
    qu_sb = [None] * NB
    qd_sb = [None] * NB
    hT = [None] * NB
    hC = [None] * NB
    col_sb = [None] * NB
    hi_sb = [None] * NB
    wk_sb = [None] * NB
    pd1_sb = [None] * NB
    pd2_sb = [None] * NB
    arct_last = [None]

    for it in range(NB + 3):
        # ---------------- stage 0: load + channel sum ----------------
        b = it
        if b < NB:
            xt = inp.tile([P, C * W], F32, tag="x")
            for c in range(C):
                nc.sync.dma_start(xt[:, c * W:(c + 1) * W],
                                  img[c, b * P:(b + 1) * P, :])
            ps_s = psA.tile([P, W], F32, tag="psA")
            conv(ps_s, [(Iw, xt[:, c * W:(c + 1) * W]) for c in range(C)])
            st = spool.tile([P, W], F32, tag="s")
            s_sb[b] = st
            sc.activation(st[:], ps_s[:], AF.Copy)
            # stage halo rows for the t/u convolutions (consumer-indexed)
            if b == 0:
                h0 = hpool.tile([2, W], F32, tag="hT")
                hT[0] = h0
                gp.memset(h0[0:2, :], 0.0)
            if b < NB - 1:
                hn = hpool.tile([2, W], F32, tag="hT")
                hT[b + 1] = hn
                gp.memset(hn[0:2, :], 0.0)
                gp.dma_start(hn[0:1, :], st[P - 1:P, :])
            if b >= 1:
                gp.dma_start(hT[b - 1][1:2, :], st[0:1, :])

        # -------- stage 1: gradients, q, orientation, q row shifts --------
        j = it - 1
        if 0 <= j < NB:
            # t (vertical smooth, prescaled /3) then gx via horizontal diff
            ps_t = psA.tile([P, W], F32, tag="psA")
            conv(ps_t, [(Vs, s_sb[j][:]), (hvT2, hT[j][:])])
            tpd = tupool.tile([P, W + 2], F32, tag="tp")
            gp.memset(tpd[:, 0:1], 0.0)
            gp.memset(tpd[:, W + 1:W + 2], 0.0)
            sc.activation(tpd[:, 1:W + 1], ps_t[:], AF.Copy)
            gxr = gout.tile([P, W], F32, tag="gx")
            v.tensor_tensor(gxr[:], tpd[:, 2:W + 2], tpd[:, 0:W],
                            ALU.subtract)
            nc.sync.dma_start(o_gx[j * P:(j + 1) * P, :], gxr[:])

            # u (vertical diff, prescaled /3) then gy via horizontal smooth
            ps_u = psA.tile([P, W], F32, tag="psA")
            conv(ps_u, [(Vd, s_sb[j][:]), (hvU2, hT[j][:])])
            upd = tupool.tile([P, W + 2], F32, tag="up")
            gp.memset(upd[:, 0:1], 0.0)
            gp.memset(upd[:, W + 1:W + 2], 0.0)
            sc.activation(upd[:, 1:W + 1], ps_u[:], AF.Copy)
            a = scr.tile([P, W], F32, tag="a")
            v.tensor_tensor(a[:], upd[:, 0:W], upd[:, 2:W + 2], ALU.add)
            gyr = gout.tile([P, W], F32, tag="gy")
            v.scalar_tensor_tensor(gyr[:], a[:], 0.5, upd[:, 1:W + 1],
                                   ALU.mult, ALU.add)
            nc.sync.dma_start(o_gy[j * P:(j + 1) * P, :], gyr[:])

            # q = gx^2 + gy^2 (padded, kept alive for NMS + sqrt phase)
            gx2 = sq.tile([P, W], F32, tag="gx2")
            sc.activation(gx2[:], gxr[:], AF.Square)
            gy2 = sq.tile([P, W], F32, tag="gy2")
            sc.activation(gy2[:], gyr[:], AF.Square)
            q = qpool.tile([P, W + 2], F32, tag="q")
            q_sb[j] = q
            gp.memset(q[:, 0:1], 0.0)
            gp.memset(q[:, W + 1:W + 2], 0.0)
            gp.tensor_tensor(q[:, 1:W + 1], gx2[:], gy2[:], ALU.add)

            # orientation: r = gy/gx (fast recip), clamp NaN, o1i, masks
            rv = scr.tile([P, W], F32, tag="rv")
            v.reciprocal_approx_fast(rv[:], gxr[:])
            r = scr.tile([P, W], F32, tag="r")
            v.tensor_tensor(r[:], gyr[:], rv[:], ALU.mult)
            rc = scr.tile([P, W], F32, tag="rv")
            v.tensor_scalar(rc[:], r[:], BIG, -BIG, ALU.min, ALU.max)
            arct = scr.tile([P, W], F32, tag="arct")
            sc.activation(arct[:], rc[:], AF.Arctan)
            if j == NB - 1:
                arct_last[0] = arct
            o1i = scr.tile([P, W], U8, tag="o1i")
            v.tensor_scalar(o1i[:], arct[:], K8PI, 4.0, ALU.mult, ALU.add)
            oro = outp.tile([P, W], BF16, tag="oro")
            sc.activation(oro[:], o1i[:], AF.Copy, scale=45.0)
            nc.sync.dma_start(o_or[j * P:(j + 1) * P, :], oro[:])
            pd1 = pdpool.tile([P, W], U8, tag="pd1")
            pd1_sb[j] = pd1
            v.tensor_scalar(pd1[:], o1i[:], 1, None, ALU.bitwise_and)
            pd2 = pdpool.tile([P, W], U8, tag="pd2")
            pd2_sb[j] = pd2
            v.tensor_scalar(pd2[:], o1i[:], 2, None, ALU.bitwise_and)

            # vertical neighbor maps via DMA partition shifts
            qu = shp.tile([P, W + 2], F32, tag="qu")
            qu_sb[j] = qu
            nc.sync.dma_start(qu[1:P, :], q[0:P - 1, :])
            if j == 0:
                nc.sync.dma_start(qu[0:1, :], zs[:])
            else:
                nc.sync.dma_start(qu[0:1, :], q_sb[j - 1][P - 1:P, :])
            qd = shp.tile([P, W + 2], F32, tag="qd")
            qd_sb[j] = qd
            nc.sync.dma_start(qd[0:P - 1, :], q[1:P, :])
            if j >= 1:
                # push this block's top row into the previous block's qd
                nc.sync.dma_start(qd_sb[j - 1][P - 1:P, :], q[0:1, :])
            if j == NB - 1:
                nc.sync.dma_start(qd[P - 1:P, :], zs[:])

        # ---------------- stage 2: NMS + thresholds ----------------
        k = it - 2
        if 0 <= k < NB:
            q = q_sb[k]
            qu = qu_sb[k]
            qd = qd_sb[k]
            M = nms.tile([P, W], F32, tag="M")
            v.tensor_tensor(M[:], q[:, 0:W], q[:, 2:W + 2], ALU.max)
            MNS = nms.tile([P, W], F32, tag="MNS")
            v.tensor_tensor(MNS[:], qu[:, 1:W + 1], qd[:, 1:W + 1], ALU.max)
            D1 = nms.tile([P, W], F32, tag="D1")
            v.tensor_tensor(D1[:], qu[:, 2:W + 2], qd[:, 0:W], ALU.max)
            D2 = nms.tile([P, W], F32, tag="D2")
            v.tensor_tensor(D2[:], qu[:, 0:W], qd[:, 2:W + 2], ALU.max)
            pd1 = pd1_sb[k]
            pd2 = pd2_sb[k]
            v.copy_predicated(D1[:], pd2[:], D2[:])
            v.copy_predicated(M[:], pd2[:], MNS[:])
            v.copy_predicated(M[:], pd1[:], D1[:])

            lo = nms.tile([P, W], BF16, tag="lo")
            v.scalar_tensor_tensor(lo[:], M[:], 0.25, q[:, 1:W + 1],
                                   ALU.max, ALU.is_lt)
            hi = rpool.tile([P, W], BF16, tag="hi")
            hi_sb[k] = hi
            v.scalar_tensor_tensor(hi[:], M[:], 1.0, q[:, 1:W + 1],
                                   ALU.max, ALU.is_lt)
            wk = rpool.tile([P, W], BF16, tag="wk")
            wk_sb[k] = wk
            v.tensor_tensor(wk[:], lo[:], hi[:], ALU.subtract)

            bt = nms.tile([P, W + 2], BF16, tag="bt")
            gp.memset(bt[:, 0:1], 0.0)
            gp.memset(bt[:, W + 1:W + 2], 0.0)
            gp.tensor_tensor(bt[:, 1:W + 1], lo[:], hi[:], ALU.add)
            ca = nms.tile([P, W], BF16, tag="ca")
            v.tensor_tensor(ca[:], bt[:, 0:W], bt[:, 2:W + 2], ALU.add)
            col = rpool.tile([P, W], BF16, tag="col")
            col_sb[k] = col
            v.tensor_tensor(col[:], ca[:], bt[:, 1:W + 1], ALU.add)
            # stage col halo rows (consumer-indexed)
            if k == 0:
                c0 = hpool.tile([2, W], BF16, tag="hC")
                hC[0] = c0
                gp.memset(c0[0:2, :], 0.0)
            if k < NB - 1:
                cn = hpool.tile([2, W], BF16, tag="hC")
                hC[k + 1] = cn
                gp.memset(cn[0:2, :], 0.0)
                gp.dma_start(cn[0:1, :], col[P - 1:P, :])
            if k >= 1:
                gp.dma_start(hC[k - 1][1:2, :], col[0:1, :])

        # ---------------- stage 3: hysteresis ----------------
        l = it - 3
        if 0 <= l < NB:
            ps_S = psS.tile([P, W], F32, tag="psS")
            conv(ps_S, [(T3, col_sb[l][:]), (hvS2, hC[l][:])])
            wh = scr.tile([P, W], BF16, tag="wh")
            v.scalar_tensor_tensor(wh[:], ps_S[:], 2.0, wk_sb[l][:],
                                   ALU.is_ge, ALU.mult)
            fin = outp.tile([P, W], BF16, tag="fin")
            v.tensor_tensor(fin[:], hi_sb[l][:], wh[:], ALU.add)
            nc.sync.dma_start(o_te[l * P:(l + 1) * P, :], fin[:])

    # ------- sqrt phase (gated behind last arctan: one table swap) -------
    zc = cpool.tile([P, 1], F32, tag="zc")
    v.tensor_scalar(zc[:], arct_last[0][:, 0:1], 0.0, None, ALU.mult)
    for j in range(NB):
        gm = outp.tile([P, W], BF16, tag="gm")
        sc.activation(gm[:], q_sb[j][:, 1:W + 1], AF.Sqrt, bias=zc[:, 0:1])
        nc.sync.dma_start(o_gm[j * P:(j + 1) * P, :], gm[:])

    ctx.close()


def _build():
    nc = bacc.Bacc()
    img = nc.declare_dram_parameter("img", [C, H, W], F32, isOutput=False)
    cw = nc.declare_dram_parameter("cw", [P, 5 * P], F32, isOutput=False)
    cwb = nc.declare_dram_parameter("cwb", [P, 2 * P], BF16, isOutput=False)
    zs = nc.declare_dram_parameter("zs", [1, W + 2], F32, isOutput=False)
    o_gx = nc.declare_dram_parameter("o_gx", [H, W], F32, isOutput=True)
    o_gy = nc.declare_dram_parameter("o_gy", [H, W], F32, isOutput=True)
    o_gm = nc.declare_dram_parameter("o_gm", [H, W], BF16, isOutput=True)
    o_or = nc.declare_dram_parameter("o_or", [H, W], BF16, isOutput=True)
    o_te = nc.declare_dram_parameter("o_te", [H, W], BF16, isOutput=True)
    with tile.TileContext(nc) as tc:
        _emit(nc, tc, img, cw, cwb, zs, o_gx, o_gy, o_gm, o_or, o_te)
    nc.finalize()
    return nc


_NC_CACHE = None


def _get_nc():
    global _NC_CACHE
    if _NC_CACHE is None:
        _NC_CACHE = _build()
    return _NC_CACHE


LAST_PROF = {}


def kernel(img: np.ndarray):
    img = np.asarray(img, np.float32)
    B = img.shape[0]
    cw = _const_weights()
    cwb = _const_weights_bf16()
    nc = _get_nc()
    zs = np.zeros((1, W + 2), np.float32)
    in_maps = [{"img": np.ascontiguousarray(img[i]), "cw": cw, "cwb": cwb,
                "zs": zs}
               for i in range(B)]
    kwargs = {}
    if os.environ.get("KTRACE") == "1":
        kwargs = dict(trace=True,
                      tmpdir=os.environ.get("KTRACE_DIR")
                      or tempfile.mkdtemp(prefix="ktrace_"))
    out = run_bass_kernel_spmd(nc, in_maps, list(range(B)), **kwargs)
    LAST_PROF["exec_time_ns"] = out.exec_time_ns
    if out.instructions_and_trace:
        LAST_PROF["insts"], LAST_PROF["trace_path"] = out.instructions_and_trace
    res = out.results
    f32 = np.float32
    gx = np.stack([res[i]["o_gx"] for i in range(B)])[:, None]
    gy = np.stack([res[i]["o_gy"] for i in range(B)])[:, None]
    gm = np.stack([np.asarray(res[i]["o_gm"], f32) for i in range(B)])[:, None]
    orient = np.stack([np.asarray(res[i]["o_or"], f32)
                       for i in range(B)])[:, None]
    edges = np.stack([np.asarray(res[i]["o_te"], f32)
                      for i in range(B)])[:, None]
    return (gx, gy, gm, orient, edges)


# revision 21
# speedup vs baseline: 1.8350x; 1.8350x over previous
"""Canny filter Bass kernel for Trainium2, data-parallel over batch on 8 cores.

Layout per core: image (3, 1024, 1024) processed in 8 row-blocks of 128 rows
(partition dim = rows, free dim = columns).  Vertical 3-tap stencils (t, u,
hysteresis box) run on the tensor engine as banded matmuls with cross-block
halo rows folded in via K=2 accumulating matmuls.  The NMS vertical neighbor
maps (q shifted by one row) are produced by SBUF-to-SBUF DMA partition
shifts instead of matmuls.  Horizontal stencils are shifted-view vector ops.
Thresholds and hysteresis gating use fused scalar_tensor_tensor ops.  The
sqrt output phase is data-gated behind the last arctan so the scalar engine
loads each activation table set exactly once.
"""

import os
import tempfile
from contextlib import ExitStack

import numpy as np
import ml_dtypes

import concourse.bacc as bacc
import concourse.tile as tile
from concourse import mybir
from concourse.bass_utils import run_bass_kernel_spmd

F32 = mybir.dt.float32
I32 = mybir.dt.int32
U8 = mybir.dt.uint8
BF16 = mybir.dt.bfloat16
AF = mybir.ActivationFunctionType
ALU = mybir.AluOpType

H = W = 1024
C = 3
NB = 8          # row blocks
P = 128         # rows per block
HALF = 512      # fp32 matmul max moving free dim
INV3 = float(np.float32(1.0) / np.float32(3.0))
K8PI = float(np.float32(8.0 / np.pi))
BIG = 1e18


def _const_weights():
    """f32 [128, 5*128]: Vs/3 | Vd/3 | I | hvT2/3 | hvU2/3.

    Vs = vertical [0.5,1,0.5] band, Vd = vertical [-1,0,1] band, I = identity
    (channel sum).  The sobel /C normalization is folded into the weights.
    hv*2 blocks live in rows 0..1: row 0 multiplies prev-block row 127 (adds
    into out row 0), row 1 multiplies next-block row 0 (adds into out row
    127).
    """
    cw = np.zeros((P, 7 * P), np.float32)

    def blk(i):
        return cw[:, i * P:(i + 1) * P]

    Vs, Vd, I = blk(0), blk(1), blk(2)
    third = np.float32(INV3)
    half3 = np.float32(0.5) * third
    for m in range(P):
        Vs[m, m] = third
        if m > 0:
            Vs[m - 1, m] = half3
            Vd[m - 1, m] = -third
            blk(5)[m - 1, m] = 1.0   # Sup: out[m] = q[m-1]
        if m < P - 1:
            Vs[m + 1, m] = half3
            Vd[m + 1, m] = third
            blk(6)[m + 1, m] = 1.0   # Sdn: out[m] = q[m+1]
        I[m, m] = 1.0
    blk(3)[0, 0] = half3       # t halo
    blk(3)[1, P - 1] = half3
    blk(4)[0, 0] = -third      # u halo
    blk(4)[1, P - 1] = third
    return cw


def _const_weights_bf16():
    """bf16 [128, 2*128]: T3 (vertical ones band) | hvS2."""
    cwb = np.zeros((P, 2 * P), np.float32)
    T3 = cwb[:, 0:P]
    for m in range(P):
        T3[m, m] = 1.0
        if m > 0:
            T3[m - 1, m] = 1.0
        if m < P - 1:
            T3[m + 1, m] = 1.0
    cwb[0, P] = 1.0          # hvS2 row 0
    cwb[1, 2 * P - 1] = 1.0  # hvS2 row 1
    return cwb.astype(ml_dtypes.bfloat16)


def _emit(nc, tc, img, cw, cwb, o_gx, o_gy, o_gm, o_or, o_te):
    v = nc.vector
    sc = nc.scalar
    te = nc.tensor
    gp = nc.gpsimd

    ctx = ExitStack()
    cpool = ctx.enter_context(tc.tile_pool(name="cp", bufs=1))
    inp = ctx.enter_context(tc.tile_pool(name="inp", bufs=1))
    spool = ctx.enter_context(tc.tile_pool(name="sp", bufs=2))
    tupool = ctx.enter_context(tc.tile_pool(name="tu", bufs=2))
    gout = ctx.enter_context(tc.tile_pool(name="go", bufs=2))
    sq = ctx.enter_context(tc.tile_pool(name="sq", bufs=1))
    scr = ctx.enter_context(tc.tile_pool(name="scr", bufs=1))
    qpool = ctx.enter_context(tc.tile_pool(name="qp", bufs=NB))
    shp = ctx.enter_context(tc.tile_pool(name="shp", bufs=2))
    nms = ctx.enter_context(tc.tile_pool(name="nms", bufs=1))
    rpool = ctx.enter_context(tc.tile_pool(name="rp", bufs=2))
    pdpool = ctx.enter_context(tc.tile_pool(name="pd", bufs=2))
    hpool = ctx.enter_context(tc.tile_pool(name="hp", bufs=3))
    outp = ctx.enter_context(tc.tile_pool(name="op", bufs=2))
    psA = ctx.enter_context(tc.tile_pool(name="psA", bufs=2, space="PSUM"))
    psUD = ctx.enter_context(tc.tile_pool(name="psUD", bufs=1, space="PSUM"))
    psS = ctx.enter_context(tc.tile_pool(name="psS", bufs=1, space="PSUM"))

    cwt = cpool.tile([P, 7 * P], F32, tag="cw")
    nc.sync.dma_start(cwt[:], cw[:])
    cwbt = cpool.tile([P, 2 * P], BF16, tag="cwb")
    nc.sync.dma_start(cwbt[:], cwb[:])

    def wblk(i, rows=P):
        return cwt[0:rows, i * P:(i + 1) * P]

    Vs, Vd, Iw = wblk(0), wblk(1), wblk(2)
    hvT2, hvU2 = wblk(3, 2), wblk(4, 2)
    Sup, Sdn = wblk(5), wblk(6)
    T3 = cwbt[:, 0:P]
    hvS2 = cwbt[0:2, P:2 * P]

    def conv(out_ps, parts):
        """Accumulating matmuls into out_ps [128, W]; parts = [(lhsT, rhs)]."""
        n = len(parts)
        for i, (lt, rh) in enumerate(parts):
            for h in (0, HALF):
                te.matmul(out_ps[:, h:h + HALF], lt, rh[:, h:h + HALF],
                          start=(i == 0), stop=(i == n - 1))

    s_sb = [None] * NB
    q_sb = [None] * NB
    qu_sb = [None] * NB
    qd_sb = [None] * NB
    hT = [None] * NB
    hC = [None] * NB
    col_sb = [None] * NB
    hi_sb = [None] * NB
    wk_sb = [None] * NB
    pd1_sb = [None] * NB
    pd2_sb = [None] * NB
    arct_last = [None]

    for it in range(NB + 3):
        # ---------------- stage 0: load + channel sum ----------------
        b = it
        if b < NB:
            xt = inp.tile([P, C * W], F32, tag="x")
            for c in range(C):
                nc.sync.dma_start(xt[:, c * W:(c + 1) * W],
                                  img[c, b * P:(b + 1) * P, :])
            ps_s = psA.tile([P, W], F32, tag="psA")
            conv(ps_s, [(Iw, xt[:, c * W:(c + 1) * W]) for c in range(C)])
            st = spool.tile([P, W], F32, tag="s")
            s_sb[b] = st
            sc.activation(st[:], ps_s[:], AF.Copy)
            # stage halo rows for the t/u convolutions (consumer-indexed)
            if b == 0:
                h0 = hpool.tile([2, W], F32, tag="hT")
                hT[0] = h0
                gp.memset(h0[0:2, :], 0.0)
            if b < NB - 1:
                hn = hpool.tile([2, W], F32, tag="hT")
                hT[b + 1] = hn
                gp.memset(hn[0:2, :], 0.0)
                gp.dma_start(hn[0:1, :], st[P - 1:P, :])
            if b >= 1:
                gp.dma_start(hT[b - 1][1:2, :], st[0:1, :])

        # -------- stage 1: gradients, q, orientation, q row shifts --------
        j = it - 1
        if 0 <= j < NB:
            # t (vertical smooth, prescaled /3) then gx via horizontal diff
            ps_t = psA.tile([P, W], F32, tag="psA")
            conv(ps_t, [(Vs, s_sb[j][:]), (hvT2, hT[j][:])])
            tpd = tupool.tile([P, W + 2], F32, tag="tp")
            gp.memset(tpd[:, 0:1], 0.0)
            gp.memset(tpd[:, W + 1:W + 2], 0.0)
            sc.activation(tpd[:, 1:W + 1], ps_t[:], AF.Copy)
            gxr = gout.tile([P, W], F32, tag="gx")
            v.tensor_tensor(gxr[:], tpd[:, 2:W + 2], tpd[:, 0:W],
                            ALU.subtract)
            nc.sync.dma_start(o_gx[j * P:(j + 1) * P, :], gxr[:])

            # u (vertical diff, prescaled /3) then gy via horizontal smooth
            ps_u = psA.tile([P, W], F32, tag="psA")
            conv(ps_u, [(Vd, s_sb[j][:]), (hvU2, hT[j][:])])
            upd = tupool.tile([P, W + 2], F32, tag="up")
            gp.memset(upd[:, 0:1], 0.0)
            gp.memset(upd[:, W + 1:W + 2], 0.0)
            sc.activation(upd[:, 1:W + 1], ps_u[:], AF.Copy)
            a = scr.tile([P, W], F32, tag="a")
            v.tensor_tensor(a[:], upd[:, 0:W], upd[:, 2:W + 2], ALU.add)
            gyr = gout.tile([P, W], F32, tag="gy")
            v.scalar_tensor_tensor(gyr[:], a[:], 0.5, upd[:, 1:W + 1],
                                   ALU.mult, ALU.add)
            nc.sync.dma_start(o_gy[j * P:(j + 1) * P, :], gyr[:])

            # q = gx^2 + gy^2 (padded, kept alive for NMS + sqrt phase)
            gx2 = sq.tile([P, W], F32, tag="gx2")
            sc.activation(gx2[:], gxr[:], AF.Square)
            gy2 = sq.tile([P, W], F32, tag="gy2")
            sc.activation(gy2[:], gyr[:], AF.Square)
            q = qpool.tile([P, W + 2], F32, tag="q")
            q_sb[j] = q
            gp.memset(q[:, 0:1], 0.0)
            gp.memset(q[:, W + 1:W + 2], 0.0)
            gp.tensor_tensor(q[:, 1:W + 1], gx2[:], gy2[:], ALU.add)

            # orientation: r = gy/gx (fast recip), clamp NaN, o1i, masks
            rv = scr.tile([P, W], F32, tag="rv")
            v.reciprocal_approx_fast(rv[:], gxr[:])
            r = scr.tile([P, W], F32, tag="r")
            v.tensor_tensor(r[:], gyr[:], rv[:], ALU.mult)
            rc = scr.tile([P, W], F32, tag="rv")
            v.tensor_scalar(rc[:], r[:], BIG, -BIG, ALU.min, ALU.max)
            arct = scr.tile([P, W], F32, tag="arct")
            sc.activation(arct[:], rc[:], AF.Arctan)
            if j == NB - 1:
                arct_last[0] = arct
            o1i = scr.tile([P, W], U8, tag="o1i")
            v.tensor_scalar(o1i[:], arct[:], K8PI, 4.0, ALU.mult, ALU.add)
            oro = outp.tile([P, W], BF16, tag="oro")
            sc.activation(oro[:], o1i[:], AF.Copy, scale=45.0)
            nc.sync.dma_start(o_or[j * P:(j + 1) * P, :], oro[:])
            pd1 = pdpool.tile([P, W], U8, tag="pd1")
            pd1_sb[j] = pd1
            v.tensor_scalar(pd1[:], o1i[:], 1, None, ALU.bitwise_and)
            pd2 = pdpool.tile([P, W], U8, tag="pd2")
            pd2_sb[j] = pd2
            v.tensor_scalar(pd2[:], o1i[:], 2, None, ALU.bitwise_and)

            # vertical neighbor maps via interior shift matmuls; the
            # cross-block boundary row rides in on a 1-row DMA push.
            ps_v = psUD.tile([P, W], F32, tag="psUD")
            conv(ps_v, [(Sup, q[:, 1:W + 1])])
            qu = shp.tile([P, W + 2], F32, tag="qu")
            qu_sb[j] = qu
            gp.memset(qu[:, 0:1], 0.0)
            gp.memset(qu[:, W + 1:W + 2], 0.0)
            sc.activation(qu[:, 1:W + 1], ps_v[:], AF.Copy)
            if j >= 1:
                nc.sync.dma_start(qu[0:1, :], q_sb[j - 1][P - 1:P, :])
            ps_w = psUD.tile([P, W], F32, tag="psUD")
            conv(ps_w, [(Sdn, q[:, 1:W + 1])])
            qd = shp.tile([P, W + 2], F32, tag="qd")
            qd_sb[j] = qd
            gp.memset(qd[:, 0:1], 0.0)
            gp.memset(qd[:, W + 1:W + 2], 0.0)
            sc.activation(qd[:, 1:W + 1], ps_w[:], AF.Copy)
            if j >= 1:
                # push this block's top row into the previous block's qd
                nc.sync.dma_start(qd_sb[j - 1][P - 1:P, :], q[0:1, :])

        # ---------------- stage 2: NMS + thresholds ----------------
        k = it - 2
        if 0 <= k < NB:
            q = q_sb[k]
            qu = qu_sb[k]
            qd = qd_sb[k]
            M = nms.tile([P, W], F32, tag="M")
            v.tensor_tensor(M[:], q[:, 0:W], q[:, 2:W + 2], ALU.max)
            MNS = nms.tile([P, W], F32, tag="MNS")
            v.tensor_tensor(MNS[:], qu[:, 1:W + 1], qd[:, 1:W + 1], ALU.max)
            D1 = nms.tile([P, W], F32, tag="D1")
            v.tensor_tensor(D1[:], qu[:, 2:W + 2], qd[:, 0:W], ALU.max)
            D2 = nms.tile([P, W], F32, tag="D2")
            v.tensor_tensor(D2[:], qu[:, 0:W], qd[:, 2:W + 2], ALU.max)
            pd1 = pd1_sb[k]
            pd2 = pd2_sb[k]
            v.copy_predicated(D1[:], pd2[:], D2[:])
            v.copy_predicated(M[:], pd2[:], MNS[:])
            v.copy_predicated(M[:], pd1[:], D1[:])

            lo = nms.tile([P, W], BF16, tag="lo")
            v.scalar_tensor_tensor(lo[:], M[:], 0.25, q[:, 1:W + 1],
                                   ALU.max, ALU.is_lt)
            hi = rpool.tile([P, W], BF16, tag="hi")
            hi_sb[k] = hi
            v.scalar_tensor_tensor(hi[:], M[:], 1.0, q[:, 1:W + 1],
                                   ALU.max, ALU.is_lt)
            wk = rpool.tile([P, W], BF16, tag="wk")
            wk_sb[k] = wk
            v.tensor_tensor(wk[:], lo[:], hi[:], ALU.subtract)

            bt = nms.tile([P, W + 2], BF16, tag="bt")
            gp.memset(bt[:, 0:1], 0.0)
            gp.memset(bt[:, W + 1:W + 2], 0.0)
            gp.tensor_tensor(bt[:, 1:W + 1], lo[:], hi[:], ALU.add)
            ca = nms.tile([P, W], BF16, tag="ca")
            v.tensor_tensor(ca[:], bt[:, 0:W], bt[:, 2:W + 2], ALU.add)
            col = rpool.tile([P, W], BF16, tag="col")
            col_sb[k] = col
            v.tensor_tensor(col[:], ca[:], bt[:, 1:W + 1], ALU.add)
            # stage col halo rows (consumer-indexed)
            if k == 0:
                c0 = hpool.tile([2, W], BF16, tag="hC")
                hC[0] = c0
                gp.memset(c0[0:2, :], 0.0)
            if k < NB - 1:
                cn = hpool.tile([2, W], BF16, tag="hC")
                hC[k + 1] = cn
                gp.memset(cn[0:2, :], 0.0)
                gp.dma_start(cn[0:1, :], col[P - 1:P, :])
            if k >= 1:
                gp.dma_start(hC[k - 1][1:2, :], col[0:1, :])

        # ---------------- stage 3: hysteresis ----------------
        l = it - 3
        if 0 <= l < NB:
            ps_S = psS.tile([P, W], F32, tag="psS")
            conv(ps_S, [(T3, col_sb[l][:]), (hvS2, hC[l][:])])
            wh = scr.tile([P, W], BF16, tag="wh")
            v.scalar_tensor_tensor(wh[:], ps_S[:], 2.0, wk_sb[l][:],
                                   ALU.is_ge, ALU.mult)
            fin = outp.tile([P, W], BF16, tag="fin")
            v.tensor_tensor(fin[:], hi_sb[l][:], wh[:], ALU.add)
            nc.sync.dma_start(o_te[l * P:(l + 1) * P, :], fin[:])

    # ------- sqrt phase (gated behind last arctan: one table swap) -------
    zc = cpool.tile([P, 1], F32, tag="zc")
    v.tensor_scalar(zc[:], arct_last[0][:, 0:1], 0.0, None, ALU.mult)
    for j in range(NB):
        gm = outp.tile([P, W], BF16, tag="gm")
        sc.activation(gm[:], q_sb[j][:, 1:W + 1], AF.Sqrt, bias=zc[:, 0:1])
        nc.sync.dma_start(o_gm[j * P:(j + 1) * P, :], gm[:])

    ctx.close()


def _build():
    nc = bacc.Bacc()
    img = nc.declare_dram_parameter("img", [C, H, W], F32, isOutput=False)
    cw = nc.declare_dram_parameter("cw", [P, 7 * P], F32, isOutput=False)
    cwb = nc.declare_dram_parameter("cwb", [P, 2 * P], BF16, isOutput=False)
    o_gx = nc.declare_dram_parameter("o_gx", [H, W], F32, isOutput=True)
    o_gy = nc.declare_dram_parameter("o_gy", [H, W], F32, isOutput=True)
    o_gm = nc.declare_dram_parameter("o_gm", [H, W], BF16, isOutput=True)
    o_or = nc.declare_dram_parameter("o_or", [H, W], BF16, isOutput=True)
    o_te = nc.declare_dram_parameter("o_te", [H, W], BF16, isOutput=True)
    with tile.TileContext(nc) as tc:
        _emit(nc, tc, img, cw, cwb, o_gx, o_gy, o_gm, o_or, o_te)
    nc.finalize()
    return nc


_NC_CACHE = None


def _get_nc():
    global _NC_CACHE
    if _NC_CACHE is None:
        _NC_CACHE = _build()
    return _NC_CACHE


LAST_PROF = {}


def kernel(img: np.ndarray):
    img = np.asarray(img, np.float32)
    B = img.shape[0]
    cw = _const_weights()
    cwb = _const_weights_bf16()
    nc = _get_nc()
    in_maps = [{"img": np.ascontiguousarray(img[i]), "cw": cw, "cwb": cwb}
               for i in range(B)]
    kwargs = {}
    if os.environ.get("KTRACE") == "1":
        kwargs = dict(trace=True,
                      tmpdir=os.environ.get("KTRACE_DIR")
                      or tempfile.mkdtemp(prefix="ktrace_"))
    out = run_bass_kernel_spmd(nc, in_maps, list(range(B)), **kwargs)
    LAST_PROF["exec_time_ns"] = out.exec_time_ns
    if out.instructions_and_trace:
        LAST_PROF["insts"], LAST_PROF["trace_path"] = out.instructions_and_trace
    res = out.results
    f32 = np.float32
    gx = np.stack([res[i]["o_gx"] for i in range(B)])[:, None]
    gy = np.stack([res[i]["o_gy"] for i in range(B)])[:, None]
    gm = np.stack([np.asarray(res[i]["o_gm"], f32) for i in range(B)])[:, None]
    orient = np.stack([np.asarray(res[i]["o_or"], f32)
                       for i in range(B)])[:, None]
    edges = np.stack([np.asarray(res[i]["o_te"], f32)
                      for i in range(B)])[:, None]
    return (gx, gy, gm, orient, edges)
